# revision 67
# baseline (speedup 1.0000x reference)
"""Causal multi-head attention (B=2, S=2048, D=1024, 16 heads of 64) on 8 TRN2
NeuronCores.

Sharding: core c -> batch b = c//4, head-group g = c%4 (4 heads = 256 model
dims per core).  Wq/Wk/Wv column-parallel, Wo row-parallel; the 4 partial
outputs per batch are summed on the host (no collectives).

TensorE streams ~106us of matmul columns, ScalarE ~100us of softmax exp (the
pacer); attention blocks (ascending j) are scalar-bound, so projection/Wo
work is chopped into short PSUM-group "atoms" and interleaved into the
attention slots.  Per slot the issue order is [scores(ki)+exp+mask,
AV(ki-2), fillers]: with st bufs=3 the score matmuls never wait (so the exp
stream stays dense) and the lag-2 AV clears the av-bank WAR from the
previous block's evac.  PSUM (8 banks): st 3x[128,512] per-head score
tiles, av 2x[65,512] (K=128 AV, ones-augmented V -> softmax denom in row
64), two alternating filler pools (2+1 banks) so atom matmuls never
serialize against their own evacs.  Input DMA runs on BOTH HWDGE queues
(sync + scalar); 10 dummy matmuls warm the PE HAM clock gate during the DMA
wait.  All wo evacs on DVE; the last Wo m-group is t-split so its t0 half
overlaps the final attention block and its t1+evac (ScalarE, idle by then)
is the only tail.  Host: out[b] = sum of 4 head-group partials (+ tail
split rows via outx) + bo.
"""

import numpy as np
import ml_dtypes

B, S, D = 2, 2048, 1024
HD = 64
NH = D // HD
N_CORES = 8
GROUPS = 4          # head-groups (tensor-parallel)
JG = D // GROUPS    # local dims per core = 256
NHL = JG // HD      # local heads = 4
KCH = D // 128      # contraction chunks for projections = 8
NKT = S // 128      # sk tiles = 16
NSB = S // 512      # 512-col blocks = 4

BF16 = ml_dtypes.bfloat16

_cached = {}


def _build():
    import concourse.bacc as bacc
    import concourse.tile as tile
    import concourse.mybir as mybir

    f32 = mybir.dt.float32
    bf16 = mybir.dt.bfloat16
    Exp = mybir.ActivationFunctionType.Exp

    nc = bacc.Bacc("TRN2", target_bir_lowering=False, debug=False,
                   num_devices=N_CORES)

    xTb = nc.dram_tensor("xTb", [NSB, 128, KCH, 512], bf16,
                         kind="ExternalInput").ap()
    wqT = nc.dram_tensor("wqT", [128, KCH, JG], bf16, kind="ExternalInput").ap()
    wkT = nc.dram_tensor("wkT", [128, KCH, JG], bf16, kind="ExternalInput").ap()
    wvT = nc.dram_tensor("wvT", [128, KCH, JG], bf16, kind="ExternalInput").ap()
    woT = nc.dram_tensor("woT", [128, 2, D], bf16, kind="ExternalInput").ap()
    bqc = nc.dram_tensor("bqc", [JG, 1], f32, kind="ExternalInput").ap()
    bkc = nc.dram_tensor("bkc", [JG, 1], f32, kind="ExternalInput").ap()
    bvb = nc.dram_tensor("bvb", [128, JG], f32, kind="ExternalInput").ap()
    maskT = nc.dram_tensor("maskT", [128, 128], bf16, kind="ExternalInput").ap()
    # rows 0..1535 of the output; the last 512 rows ship as two t-partials
    out = nc.dram_tensor("out", [S - 512, D], bf16, kind="ExternalOutput").ap()
    outx = nc.dram_tensor("outx", [2, 512, D], bf16, kind="ExternalOutput").ap()

    with tile.TileContext(nc) as tc:
        with (
            tc.tile_pool(name="const", bufs=1) as cpool,
            tc.tile_pool(name="pbig", bufs=2) as p_pool,
            tc.tile_pool(name="small", bufs=4) as small_pool,
            tc.tile_pool(name="outp", bufs=3) as out_pool,
            tc.tile_pool(name="st_ps", bufs=3, space="PSUM") as st_ps,
            tc.tile_pool(name="av_ps", bufs=2, space="PSUM") as av_ps,
            tc.tile_pool(name="pja", bufs=1, space="PSUM") as pja_ps,
            tc.tile_pool(name="pjb", bufs=1, space="PSUM") as pjb_ps,
        ):
            # ---- DMA staging on BOTH HWDGE queues (sync + scalar) so the
            # lead-in is fed at ~2x single-queue bandwidth ----
            wq_sb = cpool.tile([128, KCH, JG], bf16)
            nc.scalar.dma_start(wq_sb[:], wqT[:])
            bq_sb = cpool.tile([128, 2], f32)
            nc.scalar.dma_start(bq_sb[:], bqc.rearrange("(t p) o -> p (t o)", p=128))
            # block-major: [:, b] is one contiguous 8KB/partition DMA
            xt_all = cpool.tile([128, NSB, KCH, 512], bf16)
            # whole first x-block on sync so it isn't queued behind wq
            nc.sync.dma_start(xt_all[:, 0], xTb[0])
            wk_sb = cpool.tile([128, KCH, JG], bf16)
            nc.sync.dma_start(wk_sb[:], wkT[:])
            bk_sb = cpool.tile([128, 2], f32)
            nc.sync.dma_start(bk_sb[:], bkc.rearrange("(t p) o -> p (t o)", p=128))
            wv_sb = cpool.tile([128, KCH, JG], bf16)
            nc.scalar.dma_start(wv_sb[:], wvT[:])
            bvb_sb = cpool.tile([128, JG], f32)
            nc.scalar.dma_start(bvb_sb[:], bvb[:])
            mask_sb = cpool.tile([128, 128], bf16)
            nc.scalar.dma_start(mask_sb[:], maskT[:])
            for b in range(1, NSB):
                nc.sync.dma_start(xt_all[:, b, 0:4], xTb[b, :, 0:4])
                nc.scalar.dma_start(xt_all[:, b, 4:8], xTb[b, :, 4:8])
            wo_sb = cpool.tile([128, 2, D], bf16)
            nc.sync.dma_start(wo_sb[:], woT[:])

            qt = [cpool.tile([128, S], bf16, name=f"qt{t}") for t in range(2)]
            kt = [cpool.tile([128, S], bf16, name=f"kt{t}") for t in range(2)]
            v_all = cpool.tile([128, NKT, NHL * 65], bf16)
            nc.vector.memset(
                v_all.rearrange("p k (h c) -> p k h c", c=65)[:, :, :, 64:65], 1.0)
            po = [cpool.tile([128, S], bf16, name=f"po{t}") for t in range(2)]
            # prime the exp table load (~2.7us) before the first real exp
            warm = small_pool.tile([1, 4], f32, tag="r1")
            nc.vector.memset(warm[:], 0.0)
            nc.scalar.activation(warm[:], warm[:], Exp)
            # warm up the PE HAM clock gate (~3.4us of activity releases the
            # 1.2->2.4GHz throttle) with dummy matmuls while DMA streams in
            wmm = cpool.tile([128, 512], bf16)
            nc.vector.memset(wmm[:], 0.0)
            # span the whole ~5us DMA wait (idle >3.4us re-throttles and the
            # first projection then runs cold); the N=128 tail keeps the
            # overrun past DMA-ready under ~0.25us
            for i in range(10):
                wst = st_ps.tile([128, 512], f32, tag="st", name=f"wst{i}")
                nc.tensor.matmul(wst[:], lhsT=wmm[:, 0:128], rhs=wmm[:],
                                 start=True, stop=True)
            for i in range(8):
                wst = st_ps.tile([128, 512], f32, tag="st", name=f"wsts{i}")
                nc.tensor.matmul(wst[:, 0:128], lhsT=wmm[:, 0:128],
                                 rhs=wmm[:, 0:128], start=True, stop=True)

            # ---- filler atoms: one short PSUM group each; alternate pools ----
            psel = [0]

            def pj_tile(name):
                psel[0] ^= 1
                if psel[0]:
                    return pja_ps.tile([128, 1024], f32, tag="pja", name=name)
                return pjb_ps.tile([128, 512], f32, tag="pjb", name=name)

            def qk_atom(w_sb, b_sb, dst, t, blk):
                ps = pj_tile(f"qk{t}_{blk}")
                for k in range(KCH):
                    nc.tensor.matmul(
                        ps[:, 0:512],
                        lhsT=w_sb[:, k, 128 * t:128 * t + 128],
                        rhs=xt_all[:, blk, k, :],
                        start=(k == 0), stop=(k == KCH - 1))
                nc.vector.tensor_scalar_add(
                    dst[t][:, 512 * blk:512 * blk + 512], ps[:, 0:512],
                    b_sb[:, t:t + 1])

            def v_atom(si):
                ps = pj_tile(f"v{si}")
                for k in range(KCH):
                    nc.tensor.matmul(
                        ps[:, 0:256],
                        lhsT=xt_all[:, si // 4, k,
                                    128 * (si % 4):128 * (si % 4) + 128],
                        rhs=wv_sb[:, k, :],
                        start=(k == 0), stop=(k == KCH - 1))
                nc.vector.tensor_add(
                    v_all[:, si, :].rearrange("p (h c) -> p h c", c=65)[:, :, 0:64],
                    ps[:, 0:256].rearrange("p (h c) -> p h c", c=64),
                    bvb_sb.rearrange("p (h c) -> p h c", c=64))

            # wo m-tile: 4 accumulating MMs, one [128,1024] DVE evac, one DMA
            def wo_atom(m):
                ps = pja_ps.tile([128, 1024], f32, tag="pja", name=f"wo{m}")
                psel[0] = 1
                for t in range(2):
                    for nh2 in range(2):
                        nc.tensor.matmul(
                            ps[:, 512 * nh2:512 * nh2 + 512],
                            lhsT=po[t][:, 128 * m:128 * m + 128],
                            rhs=wo_sb[:, t, 512 * nh2:512 * nh2 + 512],
                            start=(t == 0), stop=(t == 1))
                ob = out_pool.tile([128, 1024], bf16, tag="ob", name=f"ob{m}")
                nc.vector.tensor_copy(ob[:], ps[:])
                nc.sync.dma_start(out[128 * m:128 * m + 128, :], ob[:])

            # wo for m-tiles 12..15: t-split so t0 runs during attn(1,3).
            # One MM per atom, pools and evac engines alternate so the tail
            # pipelines instead of serializing through one PSUM buffer.
            wox_stage = {}

            def wox_atom(m, t, nh2):
                ps = pj_tile(f"wox{m}_{t}_{nh2}")
                nc.tensor.matmul(
                    ps[:, 0:512],
                    lhsT=po[t][:, 128 * m:128 * m + 128],
                    rhs=wo_sb[:, t, 512 * nh2:512 * nh2 + 512],
                    start=True, stop=True)
                if (m, t) not in wox_stage:
                    wox_stage[(m, t)] = out_pool.tile(
                        [128, 1024], bf16, tag="ob", name=f"obx{m}_{t}")
                ob = wox_stage[(m, t)]
                # t0 runs while ScalarE still does exps: keep its evacs on
                # DVE; the t1 tail (ScalarE idle) alternates engines
                if t == 1 and nh2 == 1:
                    nc.scalar.copy(ob[:, 512:1024], ps[:, 0:512])
                else:
                    nc.vector.tensor_copy(ob[:, 512 * nh2:512 * nh2 + 512],
                                          ps[:, 0:512])
                if nh2 == 1:
                    nc.sync.dma_start(
                        outx[t, 128 * (m - 12):128 * (m - 12) + 128, :], ob[:])
                    del wox_stage[(m, t)]

            # ---- attention block: pair of heads x one 512-col query block ----
            def attn_block(pair, j, fillers, av_start=2, fill_by=None,
                           tail_evac=False):
                nk = 4 * (j + 1)
                if fill_by is None:
                    fill_by = nk + 2
                qt_t, kt_t = qt[pair], kt[pair]
                pt = p_pool.tile([128, NKT, 1024], bf16, tag="p",
                                 name=f"pt{pair}_{j}")
                avp = [av_ps.tile([65, 512], f32, tag="av",
                                  name=f"av{pair}_{j}_{hh}") for hh in range(2)]
                nf = len(fillers)
                fi = 0
                av_emit = 0
                for ki in range(nk + 2):
                    # scores first: with st bufs=3 their st-buffer WAR (vs
                    # exp ki-2) is already clear, so the exp pacer never waits
                    if ki < nk:
                        d = max(0, 128 * ki - 512 * j)
                        for hh in range(2):
                            sth = st_ps.tile([128, 512], f32, tag="st",
                                             name=f"st{pair}_{j}_{ki}_{hh}")
                            nc.tensor.matmul(
                                sth[:, d:512],
                                lhsT=kt_t[64 * hh:64 * hh + 64,
                                          128 * ki:128 * ki + 128],
                                rhs=qt_t[64 * hh:64 * hh + 64,
                                         512 * j + d:512 * j + 512],
                                start=True, stop=True)
                            nc.scalar.activation(
                                pt[:, ki, 512 * hh + d:512 * hh + 512],
                                sth[:, d:512], Exp)
                            if ki >= 4 * j:
                                # causal diag tile: zero where sq < sk
                                nc.vector.tensor_mul(
                                    pt[:, ki, 512 * hh + d:512 * hh + d + 128],
                                    pt[:, ki, 512 * hh + d:512 * hh + d + 128],
                                    mask_sb[:])
                    # AV lags >=2 slots behind exp; av_start delays the first
                    # emission past the previous block's evac (av-bank WAR)
                    # for blocks where that stall would block the queue
                    if ki >= av_start:
                        while av_emit <= min(ki - 2, nk - 1):
                            ka = av_emit
                            da = max(0, 128 * ka - 512 * j)
                            for hh in range(2):
                                h = 2 * pair + hh
                                nc.tensor.matmul(
                                    avp[hh][0:65, da:512],
                                    lhsT=v_all[:, ka, 65 * h:65 * h + 65],
                                    rhs=pt[:, ka,
                                           512 * hh + da:512 * hh + 512],
                                    start=(ka == 0), stop=(ka == nk - 1))
                            av_emit += 1
                    while fi < nf and fi * fill_by < nf * (ki + 1):
                        fillers[fi]()
                        fi += 1
                # evac: preoutT = avp[0:64] * (1 / avp[64]) -> po bf16
                # tail_evac (last block only): ScalarE is idle after the
                # final exp, so the l-row copies run there in parallel with
                # the DVE recip/mul chain instead of serializing on DVE
                lrows = []
                for hh in range(2):
                    lrow = small_pool.tile([1, 512], f32, tag="r1")
                    if tail_evac:
                        nc.scalar.copy(lrow[:], avp[hh][64:65, :])
                    else:
                        nc.vector.tensor_copy(lrow[:], avp[hh][64:65, :])
                    lrows.append(lrow)
                for hh in range(2):
                    rbr = small_pool.tile([1, 512], f32, tag="r1")
                    # custom-DVE ops mis-read PSUM on HW: recip from SBUF only
                    nc.vector.reciprocal_approx_fast(rbr[:], lrows[hh][:])
                    rb = small_pool.tile([64, 512], f32, tag="r64")
                    nc.gpsimd.partition_broadcast(rb[:], rbr[:])
                    nc.vector.tensor_mul(
                        po[pair][64 * hh:64 * hh + 64, 512 * j:512 * j + 512],
                        avp[hh][0:64, :], rb[:])

            # ---- schedule ----
            qk_atom(wq_sb, bq_sb, qt, 0, 0)
            qk_atom(wk_sb, bk_sb, kt, 0, 0)

            attn_block(0, 0, [
                lambda: v_atom(0), lambda: v_atom(1),
                lambda: v_atom(2), lambda: v_atom(3),
                lambda: qk_atom(wq_sb, bq_sb, qt, 0, 1),
                lambda: qk_atom(wk_sb, bk_sb, kt, 0, 1),
            ])
            attn_block(0, 1, [
                # next block's q/k deps first so (1,0) never waits on them
                lambda: qk_atom(wq_sb, bq_sb, qt, 1, 0),
                lambda: qk_atom(wk_sb, bk_sb, kt, 1, 0),
                lambda: v_atom(4), lambda: v_atom(5),
                lambda: v_atom(6), lambda: v_atom(7),
            ])
            attn_block(1, 0, [
                lambda: qk_atom(wq_sb, bq_sb, qt, 1, 1),
                lambda: qk_atom(wk_sb, bk_sb, kt, 1, 1),
            ])
            attn_block(1, 1, [
                lambda: qk_atom(wq_sb, bq_sb, qt, 0, 2),
                lambda: qk_atom(wk_sb, bk_sb, kt, 0, 2),
                lambda: wo_atom(0), lambda: wo_atom(1),
            ])
            attn_block(0, 2, [
                lambda: qk_atom(wq_sb, bq_sb, qt, 1, 2),
                lambda: qk_atom(wk_sb, bk_sb, kt, 1, 2),
                lambda: v_atom(8), lambda: v_atom(9),
                lambda: v_atom(10), lambda: v_atom(11),
                lambda: wo_atom(2), lambda: wo_atom(3),
            ], av_start=4)
            attn_block(1, 2, [
                lambda: qk_atom(wq_sb, bq_sb, qt, 0, 3),
                lambda: qk_atom(wk_sb, bk_sb, kt, 0, 3),
                lambda: wo_atom(4), lambda: wo_atom(5),
                lambda: wo_atom(6),
            ], av_start=4)
            attn_block(0, 3, [
                lambda: qk_atom(wq_sb, bq_sb, qt, 1, 3),
                lambda: qk_atom(wk_sb, bk_sb, kt, 1, 3),
                lambda: v_atom(12), lambda: v_atom(13),
                lambda: v_atom(14), lambda: v_atom(15),
                lambda: wo_atom(7), lambda: wo_atom(8),
            ], av_start=4)
            attn_block(1, 3, [
                lambda: wo_atom(9), lambda: wo_atom(10),
                lambda: wo_atom(11),
                lambda: wox_atom(12, 0, 0), lambda: wox_atom(12, 0, 1),
                lambda: wox_atom(13, 0, 0), lambda: wox_atom(13, 0, 1),
                lambda: wox_atom(14, 0, 0), lambda: wox_atom(14, 0, 1),
                lambda: wox_atom(15, 0, 0), lambda: wox_atom(15, 0, 1),
            ], av_start=8, fill_by=12, tail_evac=True)
            for m in range(12, 16):
                wox_atom(m, 1, 0)
                wox_atom(m, 1, 1)

    nc.compile()
    return nc


def _get_nc():
    if "nc" not in _cached:
        _cached["nc"] = _build()
    return _cached["nc"]


def _make_in_maps(x, Wq, bq, Wk, bk, Wv, bv, Wo):
    sc = 1.0 / np.sqrt(HD)
    tri = np.arange(128)
    mask = np.where(tri[:, None] <= tri[None, :], 1.0, 0.0).astype(BF16)
    in_maps = []
    for c in range(N_CORES):
        b, g = divmod(c, GROUPS)
        sl = slice(JG * g, JG * (g + 1))
        def tile_k(a):  # [D, M] -> [128, D//128, M] contiguous
            return np.ascontiguousarray(
                a.reshape(a.shape[0] // 128, 128, a.shape[1]).transpose(1, 0, 2))

        xt = tile_k(x[b].T.astype(BF16))  # [128, KCH, S]
        xtb = np.ascontiguousarray(
            xt.reshape(128, KCH, NSB, 512).transpose(2, 0, 1, 3))
        in_maps.append({
            "maskT": mask,
            "xTb": xtb,
            "wqT": tile_k((Wq[sl] * sc).T.astype(BF16)),
            "wkT": tile_k(Wk[sl].T.astype(BF16)),
            "wvT": tile_k(Wv[sl].T.astype(BF16)),
            "woT": tile_k(Wo[:, sl].T.astype(BF16)),
            "bqc": (bq[sl] * sc).astype(np.float32).reshape(JG, 1),
            "bkc": bk[sl].astype(np.float32).reshape(JG, 1),
            "bvb": np.broadcast_to(bv[sl].astype(np.float32), (128, JG)).copy(),
        })
    return in_maps


def _assemble(results, bo):
    full = np.empty((B, S, D), np.float32)
    for b in range(B):
        acc = np.empty((S, D), np.float32)
        r0 = results[4 * b]
        acc[0:S - 512] = r0["out"]
        acc[S - 512:] = r0["outx"][0].astype(np.float32) + r0["outx"][1]
        for g in range(1, GROUPS):
            r = results[4 * b + g]
            acc[0:S - 512] += r["out"]
            acc[S - 512:] += r["outx"][0].astype(np.float32) + r["outx"][1]
        full[b] = acc + np.asarray(bo, np.float32)[None, :]
    return full


def kernel(x, Wq, bq, Wk, bk, Wv, bv, Wo, bo, _return_results=False):
    from concourse.bass_utils import run_bass_kernel_spmd

    nc = _get_nc()
    in_maps = _make_in_maps(np.asarray(x, np.float32), np.asarray(Wq, np.float32),
                            np.asarray(bq, np.float32), np.asarray(Wk, np.float32),
                            np.asarray(bk, np.float32), np.asarray(Wv, np.float32),
                            np.asarray(bv, np.float32), np.asarray(Wo, np.float32))
    res = run_bass_kernel_spmd(nc, in_maps, core_ids=list(range(N_CORES)))
    full = _assemble(res.results, bo)
    if _return_results:
        return full, res
    return full


# revision 68
# speedup vs baseline: 1.0049x; 1.0049x over previous
"""Causal multi-head attention (B=2, S=2048, D=1024, 16 heads of 64) on 8 TRN2
NeuronCores.

Sharding: core c -> batch b = c//4, head-group g = c%4 (4 heads = 256 model
dims per core).  Wq/Wk/Wv column-parallel, Wo row-parallel; the 4 partial
outputs per batch are summed on the host (no collectives).

TensorE streams ~106us of matmul columns, ScalarE ~100us of softmax exp (the
pacer); attention blocks (ascending j) are scalar-bound, so projection/Wo
work is chopped into short PSUM-group "atoms" and interleaved into the
attention slots.  Per slot the issue order is [scores(ki)+exp+mask,
AV(ki-2), fillers]: with st bufs=3 the score matmuls never wait (so the exp
stream stays dense) and the lag-2 AV clears the av-bank WAR from the
previous block's evac.  PSUM (8 banks): st 3x[128,512] per-head score
tiles, av 2x[65,512] (K=128 AV, ones-augmented V -> softmax denom in row
64), two alternating filler pools (2+1 banks) so atom matmuls never
serialize against their own evacs.  Input DMA runs on BOTH HWDGE queues
(sync + scalar); 10 dummy matmuls warm the PE HAM clock gate during the DMA
wait.  All wo evacs on DVE; the last Wo m-group is t-split so its t0 half
overlaps the final attention block and its t1+evac (ScalarE, idle by then)
is the only tail.  Host: out[b] = sum of 4 head-group partials (+ tail
split rows via outx) + bo.
"""

import numpy as np
import ml_dtypes

B, S, D = 2, 2048, 1024
HD = 64
NH = D // HD
N_CORES = 8
GROUPS = 4          # head-groups (tensor-parallel)
JG = D // GROUPS    # local dims per core = 256
NHL = JG // HD      # local heads = 4
KCH = D // 128      # contraction chunks for projections = 8
NKT = S // 128      # sk tiles = 16
NSB = S // 512      # 512-col blocks = 4

BF16 = ml_dtypes.bfloat16

_cached = {}


def _build():
    import concourse.bacc as bacc
    import concourse.tile as tile
    import concourse.mybir as mybir

    f32 = mybir.dt.float32
    bf16 = mybir.dt.bfloat16
    Exp = mybir.ActivationFunctionType.Exp

    nc = bacc.Bacc("TRN2", target_bir_lowering=False, debug=False,
                   num_devices=N_CORES)

    xTb = nc.dram_tensor("xTb", [NSB, 128, KCH, 512], bf16,
                         kind="ExternalInput").ap()
    wqT = nc.dram_tensor("wqT", [128, KCH, JG], bf16, kind="ExternalInput").ap()
    wkT = nc.dram_tensor("wkT", [128, KCH, JG], bf16, kind="ExternalInput").ap()
    wvT = nc.dram_tensor("wvT", [128, KCH, JG], bf16, kind="ExternalInput").ap()
    woT = nc.dram_tensor("woT", [128, 2, D], bf16, kind="ExternalInput").ap()
    bqc = nc.dram_tensor("bqc", [JG, 1], f32, kind="ExternalInput").ap()
    bkc = nc.dram_tensor("bkc", [JG, 1], f32, kind="ExternalInput").ap()
    bvb = nc.dram_tensor("bvb", [128, JG], f32, kind="ExternalInput").ap()
    maskT = nc.dram_tensor("maskT", [128, 128], bf16, kind="ExternalInput").ap()
    # rows 0..1535 of the output; the last 512 rows ship as two t-partials
    out = nc.dram_tensor("out", [S - 512, D], bf16, kind="ExternalOutput").ap()
    outx = nc.dram_tensor("outx", [2, 512, D], bf16, kind="ExternalOutput").ap()

    with tile.TileContext(nc) as tc:
        with (
            tc.tile_pool(name="const", bufs=1) as cpool,
            tc.tile_pool(name="pbig", bufs=2) as p_pool,
            tc.tile_pool(name="small", bufs=4) as small_pool,
            tc.tile_pool(name="outp", bufs=3) as out_pool,
            tc.tile_pool(name="st_ps", bufs=3, space="PSUM") as st_ps,
            tc.tile_pool(name="av_ps", bufs=2, space="PSUM") as av_ps,
            tc.tile_pool(name="pja", bufs=1, space="PSUM") as pja_ps,
            tc.tile_pool(name="pjb", bufs=1, space="PSUM") as pjb_ps,
        ):
            # ---- DMA staging on BOTH HWDGE queues (sync + scalar) so the
            # lead-in is fed at ~2x single-queue bandwidth ----
            wq_sb = cpool.tile([128, KCH, JG], bf16)
            nc.scalar.dma_start(wq_sb[:], wqT[:])
            bq_sb = cpool.tile([128, 2], f32)
            nc.scalar.dma_start(bq_sb[:], bqc.rearrange("(t p) o -> p (t o)", p=128))
            # block-major: [:, b] is one contiguous 8KB/partition DMA
            xt_all = cpool.tile([128, NSB, KCH, 512], bf16)
            # whole first x-block on sync so it isn't queued behind wq
            nc.sync.dma_start(xt_all[:, 0], xTb[0])
            wk_sb = cpool.tile([128, KCH, JG], bf16)
            nc.sync.dma_start(wk_sb[:], wkT[:])
            bk_sb = cpool.tile([128, 2], f32)
            nc.sync.dma_start(bk_sb[:], bkc.rearrange("(t p) o -> p (t o)", p=128))
            wv_sb = cpool.tile([128, KCH, JG], bf16)
            nc.scalar.dma_start(wv_sb[:], wvT[:])
            bvb_sb = cpool.tile([128, JG], f32)
            nc.scalar.dma_start(bvb_sb[:], bvb[:])
            mask_sb = cpool.tile([128, 128], bf16)
            nc.scalar.dma_start(mask_sb[:], maskT[:])
            for b in range(1, NSB):
                nc.sync.dma_start(xt_all[:, b, 0:4], xTb[b, :, 0:4])
                nc.scalar.dma_start(xt_all[:, b, 4:8], xTb[b, :, 4:8])
            wo_sb = cpool.tile([128, 2, D], bf16)
            nc.sync.dma_start(wo_sb[:], woT[:])

            qt = [cpool.tile([128, S], bf16, name=f"qt{t}") for t in range(2)]
            kt = [cpool.tile([128, S], bf16, name=f"kt{t}") for t in range(2)]
            v_all = cpool.tile([128, NKT, NHL * 65], bf16)
            nc.vector.memset(
                v_all.rearrange("p k (h c) -> p k h c", c=65)[:, :, :, 64:65], 1.0)
            po = [cpool.tile([128, S], bf16, name=f"po{t}") for t in range(2)]
            # prime the exp table load (~2.7us) before the first real exp
            warm = small_pool.tile([1, 4], f32, tag="r1")
            nc.vector.memset(warm[:], 0.0)
            nc.scalar.activation(warm[:], warm[:], Exp)
            # warm up the PE HAM clock gate (~3.4us of activity releases the
            # 1.2->2.4GHz throttle) with dummy matmuls while DMA streams in
            wmm = cpool.tile([128, 512], bf16)
            nc.vector.memset(wmm[:], 0.0)
            for i in range(10):
                wst = st_ps.tile([128, 512], f32, tag="st", name=f"wst{i}")
                nc.tensor.matmul(wst[:], lhsT=wmm[:, 0:128], rhs=wmm[:],
                                 start=True, stop=True)

            # ---- filler atoms: one short PSUM group each; alternate pools ----
            psel = [0]

            def pj_tile(name):
                psel[0] ^= 1
                if psel[0]:
                    return pja_ps.tile([128, 1024], f32, tag="pja", name=name)
                return pjb_ps.tile([128, 512], f32, tag="pjb", name=name)

            def qk_atom(w_sb, b_sb, dst, t, blk):
                ps = pj_tile(f"qk{t}_{blk}")
                for k in range(KCH):
                    nc.tensor.matmul(
                        ps[:, 0:512],
                        lhsT=w_sb[:, k, 128 * t:128 * t + 128],
                        rhs=xt_all[:, blk, k, :],
                        start=(k == 0), stop=(k == KCH - 1))
                nc.vector.tensor_scalar_add(
                    dst[t][:, 512 * blk:512 * blk + 512], ps[:, 0:512],
                    b_sb[:, t:t + 1])

            def v_atom(si):
                ps = pj_tile(f"v{si}")
                for k in range(KCH):
                    nc.tensor.matmul(
                        ps[:, 0:256],
                        lhsT=xt_all[:, si // 4, k,
                                    128 * (si % 4):128 * (si % 4) + 128],
                        rhs=wv_sb[:, k, :],
                        start=(k == 0), stop=(k == KCH - 1))
                nc.vector.tensor_add(
                    v_all[:, si, :].rearrange("p (h c) -> p h c", c=65)[:, :, 0:64],
                    ps[:, 0:256].rearrange("p (h c) -> p h c", c=64),
                    bvb_sb.rearrange("p (h c) -> p h c", c=64))

            # wo m-tile: 4 accumulating MMs, one [128,1024] DVE evac, one DMA
            def wo_atom(m):
                ps = pja_ps.tile([128, 1024], f32, tag="pja", name=f"wo{m}")
                psel[0] = 1
                for t in range(2):
                    for nh2 in range(2):
                        nc.tensor.matmul(
                            ps[:, 512 * nh2:512 * nh2 + 512],
                            lhsT=po[t][:, 128 * m:128 * m + 128],
                            rhs=wo_sb[:, t, 512 * nh2:512 * nh2 + 512],
                            start=(t == 0), stop=(t == 1))
                ob = out_pool.tile([128, 1024], bf16, tag="ob", name=f"ob{m}")
                nc.vector.tensor_copy(ob[:], ps[:])
                nc.sync.dma_start(out[128 * m:128 * m + 128, :], ob[:])

            # wo for m-tiles 12..15: t-split so t0 runs during attn(1,3).
            # One MM per atom, pools and evac engines alternate so the tail
            # pipelines instead of serializing through one PSUM buffer.
            wox_stage = {}

            def wox_atom(m, t, nh2):
                ps = pj_tile(f"wox{m}_{t}_{nh2}")
                nc.tensor.matmul(
                    ps[:, 0:512],
                    lhsT=po[t][:, 128 * m:128 * m + 128],
                    rhs=wo_sb[:, t, 512 * nh2:512 * nh2 + 512],
                    start=True, stop=True)
                if (m, t) not in wox_stage:
                    wox_stage[(m, t)] = out_pool.tile(
                        [128, 1024], bf16, tag="ob", name=f"obx{m}_{t}")
                ob = wox_stage[(m, t)]
                # t0 runs while ScalarE still does exps: keep its evacs on
                # DVE; the t1 tail (ScalarE idle) alternates engines
                if t == 1 and nh2 == 1:
                    nc.scalar.copy(ob[:, 512:1024], ps[:, 0:512])
                else:
                    nc.vector.tensor_copy(ob[:, 512 * nh2:512 * nh2 + 512],
                                          ps[:, 0:512])
                if nh2 == 1:
                    nc.sync.dma_start(
                        outx[t, 128 * (m - 12):128 * (m - 12) + 128, :], ob[:])
                    del wox_stage[(m, t)]

            # ---- attention block: pair of heads x one 512-col query block ----
            def attn_block(pair, j, fillers, av_start=2, fill_by=None,
                           tail_evac=False):
                nk = 4 * (j + 1)
                if fill_by is None:
                    fill_by = nk + 2
                qt_t, kt_t = qt[pair], kt[pair]
                pt = p_pool.tile([128, NKT, 1024], bf16, tag="p",
                                 name=f"pt{pair}_{j}")
                avp = [av_ps.tile([65, 512], f32, tag="av",
                                  name=f"av{pair}_{j}_{hh}") for hh in range(2)]
                nf = len(fillers)
                fi = 0
                av_emit = 0
                for ki in range(nk + 2):
                    # scores first: with st bufs=3 their st-buffer WAR (vs
                    # exp ki-2) is already clear, so the exp pacer never waits
                    if ki < nk:
                        d = max(0, 128 * ki - 512 * j)
                        for hh in range(2):
                            sth = st_ps.tile([128, 512], f32, tag="st",
                                             name=f"st{pair}_{j}_{ki}_{hh}")
                            nc.tensor.matmul(
                                sth[:, d:512],
                                lhsT=kt_t[64 * hh:64 * hh + 64,
                                          128 * ki:128 * ki + 128],
                                rhs=qt_t[64 * hh:64 * hh + 64,
                                         512 * j + d:512 * j + 512],
                                start=True, stop=True)
                            nc.scalar.activation(
                                pt[:, ki, 512 * hh + d:512 * hh + 512],
                                sth[:, d:512], Exp)
                            if ki >= 4 * j:
                                # causal diag tile: zero where sq < sk
                                nc.vector.tensor_mul(
                                    pt[:, ki, 512 * hh + d:512 * hh + d + 128],
                                    pt[:, ki, 512 * hh + d:512 * hh + d + 128],
                                    mask_sb[:])
                    # AV lags >=2 slots behind exp; av_start delays the first
                    # emission past the previous block's evac (av-bank WAR)
                    # for blocks where that stall would block the queue
                    if ki >= av_start:
                        while av_emit <= min(ki - 2, nk - 1):
                            ka = av_emit
                            da = max(0, 128 * ka - 512 * j)
                            for hh in range(2):
                                h = 2 * pair + hh
                                nc.tensor.matmul(
                                    avp[hh][0:65, da:512],
                                    lhsT=v_all[:, ka, 65 * h:65 * h + 65],
                                    rhs=pt[:, ka,
                                           512 * hh + da:512 * hh + 512],
                                    start=(ka == 0), stop=(ka == nk - 1))
                            av_emit += 1
                    while fi < nf and fi * fill_by < nf * (ki + 1):
                        fillers[fi]()
                        fi += 1
                # evac: preoutT = avp[0:64] * (1 / avp[64]) -> po bf16
                # tail_evac (last block only): ScalarE is idle after the
                # final exp, so the l-row copies run there in parallel with
                # the DVE recip/mul chain instead of serializing on DVE
                lrows = []
                for hh in range(2):
                    lrow = small_pool.tile([1, 512], f32, tag="r1")
                    if tail_evac:
                        nc.scalar.copy(lrow[:], avp[hh][64:65, :])
                    else:
                        nc.vector.tensor_copy(lrow[:], avp[hh][64:65, :])
                    lrows.append(lrow)
                for hh in range(2):
                    rbr = small_pool.tile([1, 512], f32, tag="r1")
                    # custom-DVE ops mis-read PSUM on HW: recip from SBUF only
                    nc.vector.reciprocal_approx_fast(rbr[:], lrows[hh][:])
                    rb = small_pool.tile([64, 512], f32, tag="r64")
                    nc.gpsimd.partition_broadcast(rb[:], rbr[:])
                    nc.vector.tensor_mul(
                        po[pair][64 * hh:64 * hh + 64, 512 * j:512 * j + 512],
                        avp[hh][0:64, :], rb[:])

            # ---- schedule ----
            qk_atom(wq_sb, bq_sb, qt, 0, 0)
            qk_atom(wk_sb, bk_sb, kt, 0, 0)

            attn_block(0, 0, [
                lambda: v_atom(0), lambda: v_atom(1),
                lambda: v_atom(2), lambda: v_atom(3),
                lambda: qk_atom(wq_sb, bq_sb, qt, 0, 1),
                lambda: qk_atom(wk_sb, bk_sb, kt, 0, 1),
            ])
            attn_block(0, 1, [
                # next block's q/k deps first so (1,0) never waits on them
                lambda: qk_atom(wq_sb, bq_sb, qt, 1, 0),
                lambda: qk_atom(wk_sb, bk_sb, kt, 1, 0),
                lambda: v_atom(4), lambda: v_atom(5),
                lambda: v_atom(6), lambda: v_atom(7),
            ])
            attn_block(1, 0, [
                lambda: qk_atom(wq_sb, bq_sb, qt, 1, 1),
                lambda: qk_atom(wk_sb, bk_sb, kt, 1, 1),
            ])
            attn_block(1, 1, [
                lambda: qk_atom(wq_sb, bq_sb, qt, 0, 2),
                lambda: qk_atom(wk_sb, bk_sb, kt, 0, 2),
                lambda: wo_atom(0), lambda: wo_atom(1),
            ])
            attn_block(0, 2, [
                lambda: qk_atom(wq_sb, bq_sb, qt, 1, 2),
                lambda: qk_atom(wk_sb, bk_sb, kt, 1, 2),
                lambda: v_atom(8), lambda: v_atom(9),
                lambda: v_atom(10), lambda: v_atom(11),
                lambda: wo_atom(2), lambda: wo_atom(3),
            ], av_start=4)
            attn_block(1, 2, [
                lambda: qk_atom(wq_sb, bq_sb, qt, 0, 3),
                lambda: qk_atom(wk_sb, bk_sb, kt, 0, 3),
                lambda: wo_atom(4), lambda: wo_atom(5),
                lambda: wo_atom(6),
            ], av_start=4)
            attn_block(0, 3, [
                lambda: qk_atom(wq_sb, bq_sb, qt, 1, 3),
                lambda: qk_atom(wk_sb, bk_sb, kt, 1, 3),
                lambda: v_atom(12), lambda: v_atom(13),
                lambda: v_atom(14), lambda: v_atom(15),
                lambda: wo_atom(7), lambda: wo_atom(8),
            ], av_start=4)
            attn_block(1, 3, [
                lambda: wo_atom(9), lambda: wo_atom(10),
                lambda: wo_atom(11),
                lambda: wox_atom(12, 0, 0), lambda: wox_atom(12, 0, 1),
                lambda: wox_atom(13, 0, 0), lambda: wox_atom(13, 0, 1),
                lambda: wox_atom(14, 0, 0), lambda: wox_atom(14, 0, 1),
                lambda: wox_atom(15, 0, 0), lambda: wox_atom(15, 0, 1),
            ], av_start=8, fill_by=12, tail_evac=True)
            for m in range(12, 16):
                wox_atom(m, 1, 0)
                wox_atom(m, 1, 1)

    nc.compile()
    return nc


def _get_nc():
    if "nc" not in _cached:
        _cached["nc"] = _build()
    return _cached["nc"]


def _make_in_maps(x, Wq, bq, Wk, bk, Wv, bv, Wo):
    sc = 1.0 / np.sqrt(HD)
    tri = np.arange(128)
    mask = np.where(tri[:, None] <= tri[None, :], 1.0, 0.0).astype(BF16)
    in_maps = []
    for c in range(N_CORES):
        b, g = divmod(c, GROUPS)
        sl = slice(JG * g, JG * (g + 1))
        def tile_k(a):  # [D, M] -> [128, D//128, M] contiguous
            return np.ascontiguousarray(
                a.reshape(a.shape[0] // 128, 128, a.shape[1]).transpose(1, 0, 2))

        xt = tile_k(x[b].T.astype(BF16))  # [128, KCH, S]
        xtb = np.ascontiguousarray(
            xt.reshape(128, KCH, NSB, 512).transpose(2, 0, 1, 3))
        in_maps.append({
            "maskT": mask,
            "xTb": xtb,
            "wqT": tile_k((Wq[sl] * sc).T.astype(BF16)),
            "wkT": tile_k(Wk[sl].T.astype(BF16)),
            "wvT": tile_k(Wv[sl].T.astype(BF16)),
            "woT": tile_k(Wo[:, sl].T.astype(BF16)),
            "bqc": (bq[sl] * sc).astype(np.float32).reshape(JG, 1),
            "bkc": bk[sl].astype(np.float32).reshape(JG, 1),
            "bvb": np.broadcast_to(bv[sl].astype(np.float32), (128, JG)).copy(),
        })
    return in_maps


def _assemble(results, bo):
    full = np.empty((B, S, D), np.float32)
    for b in range(B):
        acc = np.empty((S, D), np.float32)
        r0 = results[4 * b]
        acc[0:S - 512] = r0["out"]
        acc[S - 512:] = r0["outx"][0].astype(np.float32) + r0["outx"][1]
        for g in range(1, GROUPS):
            r = results[4 * b + g]
            acc[0:S - 512] += r["out"]
            acc[S - 512:] += r["outx"][0].astype(np.float32) + r["outx"][1]
        full[b] = acc + np.asarray(bo, np.float32)[None, :]
    return full


def kernel(x, Wq, bq, Wk, bk, Wv, bv, Wo, bo, _return_results=False):
    from concourse.bass_utils import run_bass_kernel_spmd

    nc = _get_nc()
    in_maps = _make_in_maps(np.asarray(x, np.float32), np.asarray(Wq, np.float32),
                            np.asarray(bq, np.float32), np.asarray(Wk, np.float32),
                            np.asarray(bk, np.float32), np.asarray(Wv, np.float32),
                            np.asarray(bv, np.float32), np.asarray(Wo, np.float32))
    res = run_bass_kernel_spmd(nc, in_maps, core_ids=list(range(N_CORES)))
    full = _assemble(res.results, bo)
    if _return_results:
        return full, res
    return full
